# revision 13
# baseline (speedup 1.0000x reference)
"""GuidedCrossAttention Trainium2 kernel.

Sharding: 16 graphs -> 8 cores, 2 graphs per core (sorted batch indices make
graphs contiguous row-ranges). Per core we run block-diagonal attention on its
two graphs only. All projections are host-folded into single effective
matrices:
  q2 = xq @ Wq_eff + bq_eff      (SCALE folded in)
  k2 = xk @ Wk_eff + bk_eff
  v2 = xk @ Wv_eff               (v-bias folded into the residual via out-proj)
  out = ctx @ Wout_eff (+ bout folded into the residual term on host)

Device pipeline (feature-major activations so every matmul streams directly):
  q2T/k2T feature-major, v2 token-major with an appended valid-key column.
  S^T[k,q] per (graph, head, key-tile) -> exp (no max-subtract: |scores| << 1
  by construction) -> U = [v2; valid]^T @ E gives unnormalized ctx^T rows and
  the softmax denominator row in one accumulation. Normalization happens as a
  fused copy (U * broadcast(1/d)). Final projection back to token-major, then
  residual + LayerNorm.
"""

import math
from contextlib import ExitStack

import numpy as np

import concourse.bass as bass
import concourse.tile as tile
from concourse import bacc, mybir
from concourse.bass_utils import run_bass_kernel_spmd

QD, KD, HID, NH = 256, 320, 256, 8
NQ, NK, NB = 4096, 4096, 16
DH = HID // NH
EPS = 1e-5
SCALE = 1.0 / math.sqrt(DH)
NCORES = 8
GPC = NB // NCORES  # graphs per core
F32 = mybir.dt.float32
F32R = mybir.dt.float32r  # fp32 data, single-pass PE mode (4x fp32 throughput)
PASS_BARRIERS = False
PHASE_BARRIERS = False
DMA_BIG = "gpsimd"
ATTN_ON = True
NORM_ON = True
OUT_ON = True
PROJ_ON = True


def _ceil(a, b):
    return -(-a // b)


def _nsplits(total, step=512):
    return [(a, min(a + step, total)) for a in range(0, total, step)]


def _build_program(QB, KB, NQC, NQCP, KBC):
    KTC = KBC // 128  # key tiles per core (both graphs)
    KT = KB // 128  # key tiles per graph
    QT = NQCP // 128  # token-major query tiles

    nc = bacc.Bacc(
        "TRN2", target_bir_lowering=False, debug=False, num_devices=NCORES
    )
    xqT_d = nc.declare_dram_parameter("xqT", [QD, NQC], F32R, isOutput=False)
    xqtok_d = nc.declare_dram_parameter("xqtok", [NQCP, QD], F32, isOutput=False)
    xkT_d = nc.declare_dram_parameter("xkT", [KD + 1, KBC], F32R, isOutput=False)
    wq_d = nc.declare_dram_parameter("wq", [QD, 384], F32R, isOutput=False)
    wk_d = nc.declare_dram_parameter("wk", [KD + 1, 384], F32R, isOutput=False)
    wv_d = nc.declare_dram_parameter("wv", [KD + 1, NH * 2 * DH], F32R, isOutput=False)
    wo_d = nc.declare_dram_parameter("wo", [HID, QD], F32R, isOutput=False)
    bq_d = nc.declare_dram_parameter("bq", [384], F32, isOutput=False)
    bk_d = nc.declare_dram_parameter("bk", [384], F32, isOutput=False)
    lng_d = nc.declare_dram_parameter("lng", [QD], F32, isOutput=False)
    lnb_d = nc.declare_dram_parameter("lnb", [QD], F32, isOutput=False)
    out_d = nc.declare_dram_parameter("out", [NQCP, QD], F32, isOutput=True)

    kchunks = [(0, 128), (128, 256), (256, KD + 1)]
    _barrier_noop = lambda: None  # KD+1=321 partition chunks (valid row)

    with tile.TileContext(nc) as tc, ExitStack() as ctx:
        _dma_big = getattr(nc, DMA_BIG)
        _pass_bar = tc.strict_bb_all_engine_barrier if PASS_BARRIERS else _barrier_noop
        _phase_bar = tc.strict_bb_all_engine_barrier if PHASE_BARRIERS else _barrier_noop
        P = ctx.enter_context(tc.tile_pool(name="persist", bufs=1))

        # ---- constant / activation loads ----
        xqT = [P.tile([128, NQC], F32R, tag=f"xqT{i}", name=f"xqT{i}") for i in range(2)]
        for i in range(2):
            _dma_big.dma_start(out=xqT[i], in_=xqT_d[128 * i : 128 * (i + 1), :])
        xkT = []
        for i, (a, b) in enumerate(kchunks):
            t = P.tile([b - a, KBC], F32R, tag=f"xkT{i}", name=f"xkT{i}")
            xkT.append(t)
            _dma_big.dma_start(out=t, in_=xkT_d[a:b, :])
        wq = [P.tile([128, 384], F32R, tag=f"wq{i}", name=f"wq{i}") for i in range(2)]
        wo = [P.tile([32, QD], F32R, tag=f"wo{i}", name=f"wo{i}") for i in range(NH)]
        for i in range(2):
            _dma_big.dma_start(out=wq[i], in_=wq_d[128 * i : 128 * (i + 1), :])
        for i in range(NH):
            _dma_big.dma_start(out=wo[i], in_=wo_d[DH * i : DH * (i + 1), :])
        wk, wv = [], []
        for i, (a, b) in enumerate(kchunks):
            tk = P.tile([b - a, 384], F32R, tag=f"wk{i}")
            tv = P.tile([b - a, NH * 2 * DH], F32R, tag=f"wv{i}")
            wk.append(tk)
            wv.append(tv)
            _dma_big.dma_start(out=tk, in_=wk_d[a:b, :])
            _dma_big.dma_start(out=tv, in_=wv_d[a:b, :])
        bq = [P.tile([128, 1], F32, tag=f"bq{i}", name=f"bq{i}") for i in range(3)]
        bk = [P.tile([128, 1], F32, tag=f"bk{i}", name=f"bk{i}") for i in range(3)]
        for i in range(3):
            nc.gpsimd.dma_start(out=bq[i], in_=bq_d[128 * i : 128 * (i + 1)])
            nc.gpsimd.dma_start(out=bk[i], in_=bk_d[128 * i : 128 * (i + 1)])
        lng = P.tile([128, QD], F32, tag="lng")
        lnb = P.tile([128, QD], F32, tag="lnb")
        nc.sync.dma_start(
            out=lng,
            in_=bass.AP(
                tensor=lng_d.ap().tensor, offset=0, ap=[[0, 128], [1, QD]]
            ),
        )
        nc.sync.dma_start(
            out=lnb,
            in_=bass.AP(
                tensor=lnb_d.ap().tensor, offset=0, ap=[[0, 128], [1, QD]]
            ),
        )
        epst = P.tile([128, 1], F32, tag="epst")
        nc.vector.memset(epst, EPS)
        xqtok = [P.tile([128, QD], F32, tag=f"xqtok{i}", name=f"xqtok{i}") for i in range(QT)]
        for i in range(QT):
            _dma_big.dma_start(out=xqtok[i], in_=xqtok_d[128 * i : 128 * (i + 1), :])

        q2T = [P.tile([128, NQC], F32R, tag=f"q2T{i}", name=f"q2T{i}") for i in range(3)]
        k2T = [P.tile([128, KBC], F32R, tag=f"k2T{i}", name=f"k2T{i}") for i in range(3)]
        v2e = [P.tile([128, NH, 2 * DH], F32R, tag=f"v2e{i}", name=f"v2e{i}") for i in range(KTC)]
        ctxT = [P.tile([32, NQCP], F32R, tag=f"ctxT{i}", name=f"ctxT{i}") for i in range(NH)]
        # no memset: pad columns (NQC..NQCP) feed only pad token rows, discarded on host

        # ---- projections ----
        with tc.tile_pool(name="proj_ps", bufs=2, space="PSUM") as pp:
            with tc.tile_pool(name="junk_ps", bufs=1, space="PSUM") as jp:
                junk = jp.tile([1, 1], F32, tag="junk", name="junk")
                for t in [xqT[0], xqT[1], *xkT, *wq, *wk, *wv]:
                    nc.tensor.matmul(
                        junk,
                        lhsT=t[0:1, 0:1].bitcast(F32),
                        rhs=epst[0:1, 0:1],
                        start=True,
                        stop=True,
                        skip_group_check=True,
                    )
            for mc in (range(3) if PROJ_ON else []):
                ps = pp.tile([128, NQC], F32, tag="qk_ps")
                for kc in range(2):
                    for n0, n1 in _nsplits(NQC):
                        nc.tensor.matmul(
                            ps[:, n0:n1],
                            lhsT=wq[kc][:, 128 * mc : 128 * (mc + 1)],
                            rhs=xqT[kc][:, n0:n1],
                            start=(kc == 0),
                            stop=(kc == 1),
                        )
                nc.vector.tensor_scalar(
                    out=q2T[mc],
                    in0=ps,
                    scalar1=bq[mc][:, 0:1],
                    scalar2=None,
                    op0=mybir.AluOpType.add,
                )
            for mc in (range(3) if PROJ_ON else []):
                ps = pp.tile([128, KBC], F32, tag="qk_ps")
                for kc in range(3):
                    for n0, n1 in _nsplits(KBC):
                        nc.tensor.matmul(
                            ps[:, n0:n1],
                            lhsT=wk[kc][:, 128 * mc : 128 * (mc + 1)],
                            rhs=xkT[kc][:, n0:n1],
                            start=(kc == 0),
                            stop=(kc == 2),
                        )
                nc.vector.tensor_scalar(
                    out=k2T[mc],
                    in0=ps,
                    scalar1=bk[mc][:, 0:1],
                    scalar2=None,
                    op0=mybir.AluOpType.add,
                )
            for kt in (range(KTC) if PROJ_ON else []):
                ps = pp.tile([128, NH * 2 * DH], F32, tag="v_ps")
                for kc in range(3):
                    nc.tensor.matmul(
                        ps,
                        lhsT=xkT[kc][:, 128 * kt : 128 * (kt + 1)],
                        rhs=wv[kc],
                        start=(kc == 0),
                        stop=(kc == 2),
                    )
                nc.vector.tensor_copy(
                    out=v2e[kt].rearrange("p h d -> p (h d)"),
                    in_=ps,
                )

        # ---- attention: per (graph, half-of-heads) pass ----
        with (
            tc.tile_pool(name="s_ps", bufs=2, space="PSUM") as sp,
            tc.tile_pool(name="u_ps", bufs=4, space="PSUM") as up,
            tc.tile_pool(name="e_sb", bufs=3) as ep,
            tc.tile_pool(name="d_sb", bufs=2) as dp,
        ):
            passes = [(g, half) for g in range(GPC) for half in range(2)]
            prev_pass = {}
            for pi, (g, half) in enumerate(passes if ATTN_ON else []):
                    Us = [up.tile([2 * DH, 512], F32, tag="U", name="U") for _ in range(4)]
                    if pi > 0:
                        pg, ph = passes[pi - 1]
                        for j in range(4):
                            hprev = ph * 4 + j
                            nc.tensor.matmul(
                                Us[j][0:1, 0:1],
                                lhsT=ctxT[hprev][0:1, pg * QB : pg * QB + 1].bitcast(F32),
                                rhs=epst[0:1, 0:1],
                                start=True,
                                stop=True,
                                skip_group_check=True,
                            )
                    for hp in range(2):
                        for kt in range(KT):
                            S = sp.tile([128, 2, 512], F32, tag="S")
                            for j2 in range(2):
                                h = half * 4 + hp * 2 + j2
                                mc, r = h // 3, (h % 3) * DH
                                nc.tensor.matmul(
                                    S[:, j2, 0:QB],
                                    lhsT=k2T[mc][
                                        r : r + DH,
                                        g * KB + 128 * kt : g * KB + 128 * (kt + 1),
                                    ],
                                    rhs=q2T[mc][r : r + DH, g * QB : (g + 1) * QB],
                                    start=True,
                                    stop=True,
                                )
                            E = ep.tile([128, 2, QB], F32R, tag="E")
                            nc.scalar.activation(
                                out=E,
                                in_=S[:, :, 0:QB],
                                func=mybir.ActivationFunctionType.Exp,
                            )
                            for j2 in range(2):
                                j = hp * 2 + j2
                                nc.tensor.matmul(
                                    Us[j][:, 0:QB],
                                    lhsT=v2e[g * KT + kt][:, half * 4 + j, :],
                                    rhs=E[:, j2, :],
                                    start=(kt == 0),
                                    stop=(kt == KT - 1),
                                )
                    for j in (range(4) if NORM_ON else []):
                        h = half * 4 + j
                        rd = dp.tile([DH, QB], F32, tag=f"rd{j}", name=f"rd{j}")
                        nc.scalar.activation(
                            out=rd,
                            in_=Us[j][DH : 2 * DH, 0:QB],
                            func=mybir.ActivationFunctionType.Copy,
                        )
                        rr = dp.tile([DH, QB], F32, tag=f"rr{j}", name=f"rr{j}")
                        nc.vector.reciprocal(out=rr, in_=rd)
                        nc.vector.scalar_tensor_tensor(
                            out=ctxT[h][:, g * QB : (g + 1) * QB],
                            in0=Us[j][0:DH, 0:QB],
                            scalar=0.0,
                            in1=rr,
                            op0=mybir.AluOpType.bypass,
                            op1=mybir.AluOpType.mult,
                        )


        # ---- out-projection + residual + layernorm ----
        with (
            tc.tile_pool(name="o_ps", bufs=2, space="PSUM") as op,
            tc.tile_pool(name="ln_sb", bufs=3) as lp,
        ):
            for qt in (range(QT) if OUT_ON else []):
                ps = op.tile([128, QD], F32, tag="o_ps")
                for h in range(NH):
                    if h == 0:
                        nc.tensor.matmul(
                            ps[0:1, 0:1],
                            lhsT=ctxT[7][0:1, 128 * qt : 128 * qt + 1].bitcast(F32),
                            rhs=epst[0:1, 0:1],
                            start=True,
                            stop=True,
                            skip_group_check=True,
                        )
                    nc.tensor.matmul(
                        ps,
                        lhsT=ctxT[h][:, 128 * qt : 128 * (qt + 1)],
                        rhs=wo[h],
                        start=(h == 0),
                        stop=(h == NH - 1),
                    )
                x = lp.tile([128, QD], F32, tag="x")
                nc.vector.tensor_add(x, ps, xqtok[qt])
                stats = lp.tile([128, 6], F32, tag="stats")
                nc.vector.bn_stats(out=stats, in_=x)
                mv = lp.tile([128, 2], F32, tag="mv")
                nc.vector.bn_aggr(out=mv, in_=stats)
                sd = lp.tile([128, 1], F32, tag="sd")
                nc.scalar.activation(
                    out=sd, in_=mv[:, 1:2], func=mybir.ActivationFunctionType.Sqrt,
                    bias=epst[:, 0:1],
                )
                rstd = lp.tile([128, 1], F32, tag="rstd")
                nc.vector.reciprocal(out=rstd, in_=sd)
                xc = lp.tile([128, QD], F32, tag="xc")
                nc.vector.tensor_scalar(
                    out=xc,
                    in0=x,
                    scalar1=mv[:, 0:1],
                    scalar2=None,
                    op0=mybir.AluOpType.subtract,
                )
                y = lp.tile([128, QD], F32, tag="y")
                nc.vector.scalar_tensor_tensor(
                    out=y,
                    in0=xc,
                    scalar=rstd[:, 0:1],
                    in1=lng,
                    op0=mybir.AluOpType.mult,
                    op1=mybir.AluOpType.mult,
                )
                yb = lp.tile([128, QD], F32, tag="yb")
                nc.gpsimd.tensor_add(yb, y, lnb)
                nc.sync.dma_start(out=out_d[128 * qt : 128 * (qt + 1), :], in_=yb)

    nc.compile()
    return nc


def kernel(**inputs):
    xq = np.ascontiguousarray(np.asarray(inputs["query_nodes"], dtype=np.float32))
    xk = np.ascontiguousarray(np.asarray(inputs["key_nodes"], dtype=np.float32))
    qbi = np.asarray(inputs["query_batch_idx"]).astype(np.int64)
    kbi = np.asarray(inputs["key_batch_idx"]).astype(np.int64)
    Wq = np.asarray(inputs["Wq"], np.float32)
    Wk = np.asarray(inputs["Wk"], np.float32)
    Wv = np.asarray(inputs["Wv"], np.float32)
    bq0 = np.asarray(inputs["bq"], np.float32)
    bk0 = np.asarray(inputs["bk"], np.float32)
    bv0 = np.asarray(inputs["bv"], np.float32)
    W2 = np.asarray(inputs["in_proj_w"], np.float32)
    b2 = np.asarray(inputs["in_proj_b"], np.float32)
    mow = np.asarray(inputs["mha_ow"], np.float32)
    mob = np.asarray(inputs["mha_ob"], np.float32)
    Wo = np.asarray(inputs["Wo"], np.float32)
    bo = np.asarray(inputs["bo"], np.float32)
    lng = np.asarray(inputs["ln_g"], np.float32)
    lnb = np.asarray(inputs["ln_b"], np.float32)

    # host-side weight folding
    Wq_eff = (Wq @ W2[:HID].T) * SCALE
    bq_eff = (bq0 @ W2[:HID].T + b2[:HID]) * SCALE
    Wk_eff = Wk @ W2[HID : 2 * HID].T
    bk_eff = bk0 @ W2[HID : 2 * HID].T + b2[HID : 2 * HID]
    Wv_eff = Wv @ W2[2 * HID :].T
    bv_eff = bv0 @ W2[2 * HID :].T + b2[2 * HID :]
    Wout_eff = mow @ Wo
    bout = bv_eff @ Wout_eff + mob @ Wo + bo  # folded into residual

    qcnt = np.bincount(qbi, minlength=NB)
    kcnt = np.bincount(kbi, minlength=NB)
    qoff = np.concatenate([[0], np.cumsum(qcnt)])
    koff = np.concatenate([[0], np.cumsum(kcnt)])

    QB = int(_ceil(max(int(qcnt.max()), 8), 8) * 8)
    KB = int(_ceil(max(int(kcnt.max()), 1), 128) * 128)
    NQC = GPC * QB
    NQCP = _ceil(NQC, 128) * 128
    KBC = GPC * KB

    nc = _build_program(QB, KB, NQC, NQCP, KBC)

    # pack 8 heads as 3-per-128-partition-tile (PE base-partition must be 0/32/64)
    def _headpack_cols(W):
        Wp = np.zeros((W.shape[0], 384), np.float32)
        for h in range(NH):
            Wp[:, 128 * (h // 3) + DH * (h % 3) : 128 * (h // 3) + DH * (h % 3) + DH] = (
                W[:, DH * h : DH * (h + 1)]
            )
        return Wp

    def _headpack_vec(v):
        vp = np.zeros((384,), np.float32)
        for h in range(NH):
            vp[128 * (h // 3) + DH * (h % 3) : 128 * (h // 3) + DH * (h % 3) + DH] = v[
                DH * h : DH * (h + 1)
            ]
        return vp

    wqT = _headpack_cols(Wq_eff)
    wkT = np.zeros((KD + 1, 384), np.float32)
    wkT[:KD] = _headpack_cols(Wk_eff)
    wvT = np.zeros((KD + 1, NH * 2 * DH), np.float32)
    for h in range(NH):
        wvT[:KD, 2 * DH * h : 2 * DH * h + DH] = Wv_eff[:, DH * h : DH * (h + 1)]
        wvT[KD, 2 * DH * h + DH : 2 * DH * (h + 1)] = 1.0
    woT = np.ascontiguousarray(Wout_eff)
    bq_eff = _headpack_vec(bq_eff)
    bk_eff = _headpack_vec(bk_eff)

    in_maps = []
    for c in range(NCORES):
        xqT = np.zeros((QD, NQC), np.float32)
        xqtok = np.zeros((NQCP, QD), np.float32)
        xkT = np.zeros((KD + 1, KBC), np.float32)
        for gi in range(GPC):
            g = GPC * c + gi
            nq = int(qcnt[g])
            nk = int(kcnt[g])
            if nq:
                rows = xq[qoff[g] : qoff[g + 1]]
                xqT[:, gi * QB : gi * QB + nq] = rows.T
                xqtok[gi * QB : gi * QB + nq] = rows + bout
            if nk:
                xkT[:KD, gi * KB : gi * KB + nk] = xk[koff[g] : koff[g + 1]].T
                xkT[KD, gi * KB : gi * KB + nk] = 1.0
        in_maps.append(
            {
                "xqT": xqT,
                "xqtok": xqtok,
                "xkT": xkT,
                "wq": wqT,
                "wk": wkT,
                "wv": wvT,
                "wo": woT,
                "bq": bq_eff.copy(),
                "bk": bk_eff.copy(),
                "lng": lng.copy(),
                "lnb": lnb.copy(),
            }
        )

    import os

    trace = bool(os.environ.get("BASS_TRACE"))
    res = run_bass_kernel_spmd(nc, in_maps, list(range(NCORES)), trace=trace)
    if getattr(res, "exec_time_ns", None):
        print(f"HW exec time: {res.exec_time_ns} ns")
    out = np.empty((NQ, QD), np.float32)
    for c in range(NCORES):
        oc = res.results[c]["out"]
        for gi in range(GPC):
            g = GPC * c + gi
            nq = int(qcnt[g])
            if nq:
                out[qoff[g] : qoff[g + 1]] = oc[gi * QB : gi * QB + nq]
    return out



# revision 19
# speedup vs baseline: 1.0117x; 1.0117x over previous
"""GuidedCrossAttention Trainium2 kernel.

Sharding: 16 graphs -> 8 cores, 2 graphs per core (sorted batch indices make
graphs contiguous row-ranges). Per core we run block-diagonal attention on its
two graphs only. All projections are host-folded into single effective
matrices:
  q2 = xq @ Wq_eff + bq_eff      (SCALE folded in)
  k2 = xk @ Wk_eff + bk_eff
  v2 = xk @ Wv_eff               (v-bias folded into the residual via out-proj)
  out = ctx @ Wout_eff (+ bout folded into the residual term on host)

Device pipeline (feature-major activations, float32r matmuls throughout):
  - all inputs packed host-side into one DMA per tensor, triggers spread
    over sync/scalar/vector/gpsimd so compute starts as soon as possible
  - per-(mc, graph) q2T/k2T tiles so attention on graph 0 starts while
    graph 1 still projects
  - v2 projected with per-head [V(32) | valid-replicated(32)] columns, so the
    U matmul drops both the unnormalized context AND a 32-row broadcast of
    the softmax denominator into PSUM
  - per pass (graph, half-of-heads): all S^T matmuls first, exp per key-tile
    on scalar engine, then the U accumulation chain -> tensor never waits on
    the scalar engine in steady state
  - normalization: one full-tile vector reciprocal per U-pair tile + one
    STT multiply per head into packed ctx4 tiles ([128, NQCP], 4 heads each)
  - out-projection: K=128 matmuls (2 per token tile) using ctx4 directly
  - LayerNorm rstd via exp(-0.5*ln(var+eps)) to stay in the exp/ln ACT table
    set (no mid-kernel table switch)
"""

import math

import numpy as np

import concourse.bass as bass
import concourse.tile as tile
from concourse import bacc, mybir
from concourse.bass_utils import run_bass_kernel_spmd

QD, KD, HID, NH = 256, 320, 256, 8
NQ, NK, NB = 4096, 4096, 16
DH = HID // NH
EPS = 1e-5
SCALE = 1.0 / math.sqrt(DH)
NCORES = 8
GPC = NB // NCORES  # graphs per core
F32 = mybir.dt.float32
F32R = mybir.dt.float32r  # fp32 data, single-pass PE mode (4x fp32 throughput)
AF = mybir.ActivationFunctionType
ALU = mybir.AluOpType


def _ceil(a, b):
    return -(-a // b)


def _build_program(QB, KB, NQC, NQCP, KBC):
    KT = KB // 128  # key tiles per graph
    QT = NQCP // 128  # token-major query tiles

    nc = bacc.Bacc(
        "TRN2", target_bir_lowering=False, debug=False, num_devices=NCORES
    )
    xq_d = nc.declare_dram_parameter("xq", [128, 2 * NQC], F32R, isOutput=False)
    xk_d = nc.declare_dram_parameter("xk", [128, 3 * KBC], F32R, isOutput=False)
    wq_d = nc.declare_dram_parameter("wq", [128, 2 * 384], F32R, isOutput=False)
    wk_d = nc.declare_dram_parameter("wk", [128, 3 * 384], F32R, isOutput=False)
    wv_d = nc.declare_dram_parameter("wv", [128, 3 * NH * 2 * DH], F32R, isOutput=False)
    wo_d = nc.declare_dram_parameter("wo", [128, 2 * QD], F32R, isOutput=False)
    xqtok_d = nc.declare_dram_parameter("xqtok", [128, QT * QD], F32, isOutput=False)
    bqk_d = nc.declare_dram_parameter("bqk", [128, 6], F32, isOutput=False)
    ln_d = nc.declare_dram_parameter("ln", [2 * QD], F32, isOutput=False)
    out_d = nc.declare_dram_parameter("out", [NQCP, QD], F32, isOutput=True)

    kchunks = [(0, 128), (128, 256), (256, KD + 1)]  # xk row chunks (last: 65)

    with tile.TileContext(nc) as tc:
        persist_cm = tc.tile_pool(name="persist", bufs=1)
        P = persist_cm.__enter__()

        # ---- input loads: one DMA per tensor, spread across engines ----
        xqT = P.tile([128, 2, NQC], F32R, tag="xqT", name="xqT")
        wqt = P.tile([128, 2, 384], F32R, tag="wqt", name="wqt")
        bqk = P.tile([128, 6], F32, tag="bqk", name="bqk")
        nc.sync.dma_start(out=xqT, in_=xq_d[:, :])
        nc.sync.dma_start(out=wqt, in_=wq_d[:, :])
        nc.sync.dma_start(out=bqk, in_=bqk_d[:, :])
        xkT = P.tile([128, 3, KBC], F32R, tag="xkT", name="xkT")
        wkt = P.tile([128, 3, 384], F32R, tag="wkt", name="wkt")
        nc.scalar.dma_start(out=xkT, in_=xk_d[:, :])
        nc.scalar.dma_start(out=wkt, in_=wk_d[:, :])
        wvt = P.tile([128, 3, NH * 2 * DH], F32R, tag="wvt", name="wvt")
        lnt = P.tile([128, 2, QD], F32, tag="lnt", name="lnt")
        nc.scalar.dma_start(out=wvt, in_=wv_d[:, :])
        nc.gpsimd.dma_start(
            out=lnt,
            in_=bass.AP(tensor=ln_d.ap().tensor, offset=0, ap=[[0, 128], [1, 2 * QD]]),
        )
        wo4 = P.tile([128, 2, QD], F32R, tag="wo4", name="wo4")
        xqtok = P.tile([128, QT, QD], F32, tag="xqtok", name="xqtok")
        nc.gpsimd.dma_start(out=wo4, in_=wo_d[:, :])
        nc.gpsimd.dma_start(out=xqtok, in_=xqtok_d[:, :])

        epst = P.tile([128, 1], F32, tag="epst")
        nc.vector.memset(epst, EPS)

        # per (mc, graph) projection outputs; per key-tile v2e; packed ctx4
        q2T = [
            [P.tile([128, QB], F32R, tag=f"q2T{mc}{g}", name=f"q2T{mc}{g}") for g in range(GPC)]
            for mc in range(3)
        ]
        k2T = [
            [P.tile([128, KB], F32R, tag=f"k2T{mc}{g}", name=f"k2T{mc}{g}") for g in range(GPC)]
            for mc in range(3)
        ]
        v2e = [
            P.tile([128, NH, 2 * DH], F32R, tag=f"v2e{i}", name=f"v2e{i}")
            for i in range(GPC * KT)
        ]
        ctx4 = [P.tile([128, NQCP], F32R, tag=f"ctx4{i}", name=f"ctx4{i}") for i in range(2)]

        pp_cm = tc.tile_pool(name="proj_ps", bufs=2, space="PSUM")
        pp = pp_cm.__enter__()

        junked = set()

        def _junk(ps, *tiles):
            # 1x1 matmul reading each tile once: orders DMA completion
            # ahead of the next accumulation-group start
            for t in tiles:
                if id(t) in junked:
                    continue
                junked.add(id(t))
                src = t[tuple(slice(0, 1) for _ in range(len(t.shape)))]
                nc.tensor.matmul(
                    ps[0:1, 0:1],
                    lhsT=src.bitcast(F32) if src.dtype == F32R else src,
                    rhs=epst[0:1, 0:1],
                    start=True,
                    stop=True,
                    skip_group_check=True,
                )

        def _proj_q(mc, g):
            ps = pp.tile([128, 512], F32, tag="pj_ps")
            _junk(ps, xqT, wqt)
            for kc in range(2):
                nc.tensor.matmul(
                    ps[:, 0:QB],
                    lhsT=wqt[:, kc, 128 * mc : 128 * (mc + 1)],
                    rhs=xqT[:, kc, g * QB : (g + 1) * QB],
                    start=(kc == 0),
                    stop=(kc == 1),
                )
            nc.vector.tensor_scalar(
                out=q2T[mc][g],
                in0=ps[:, 0:QB],
                scalar1=bqk[:, mc : mc + 1],
                scalar2=None,
                op0=ALU.add,
            )

        def _proj_k(mc, g):
            ps = pp.tile([128, 512], F32, tag="pj_ps")
            _junk(ps, xkT, wkt)
            for kc, (a, b) in enumerate(kchunks):
                nc.tensor.matmul(
                    ps[:, 0:KB],
                    lhsT=wkt[0 : b - a, kc, 128 * mc : 128 * (mc + 1)],
                    rhs=xkT[0 : b - a, kc, g * KB : (g + 1) * KB],
                    start=(kc == 0),
                    stop=(kc == 2),
                )
            nc.vector.tensor_scalar(
                out=k2T[mc][g],
                in0=ps[:, 0:KB],
                scalar1=bqk[:, 3 + mc : 4 + mc],
                scalar2=None,
                op0=ALU.add,
            )

        def _proj_v(kt):
            ps = pp.tile([128, 512], F32, tag="pj_ps")
            _junk(ps, wvt)
            for kc, (a, b) in enumerate(kchunks):
                nc.tensor.matmul(
                    ps,
                    lhsT=xkT[0 : b - a, kc, 128 * kt : 128 * (kt + 1)],
                    rhs=wvt[0 : b - a, kc, :],
                    start=(kc == 0),
                    stop=(kc == 2),
                )
            nc.vector.tensor_copy(
                out=v2e[kt].rearrange("p h d -> p (h d)"),
                in_=ps,
            )

        passes = [(g, half) for g in range(GPC) for half in range(2)]

        def _pass(pi):
            g, half = passes[pi]
            U4 = [
                up.tile([64, 2, 512], F32, tag=f"U4{t}", name=f"U4{t}")
                for t in range(2)
            ]
            if pi > 0:
                pg, ph = passes[pi - 1]
                for t in range(2):
                    # WAR gate: new U group waits for prev pass's ctx4 writes
                    nc.tensor.matmul(
                        U4[t][0:1, 0, 0:1],
                        lhsT=ctx4[ph][0:1, pg * QB : pg * QB + 1].bitcast(F32),
                        rhs=epst[0:1, 0:1],
                        start=True,
                        stop=True,
                        skip_group_check=True,
                    )
            for hp in range(2):
                Es = []
                for kt in range(KT):
                    S = sp.tile([128, 2, 512], F32, tag="S")
                    for j2 in range(2):
                        h = half * 4 + hp * 2 + j2
                        mc, r = h // 3, (h % 3) * DH
                        nc.tensor.matmul(
                            S[:, j2, 0:QB],
                            lhsT=k2T[mc][g][r : r + DH, 128 * kt : 128 * (kt + 1)],
                            rhs=q2T[mc][g][r : r + DH, :],
                            start=True,
                            stop=True,
                        )
                    E = ep.tile([128, 2, QB], F32R, tag="E")
                    nc.scalar.activation(out=E, in_=S[:, :, 0:QB], func=AF.Exp)
                    Es.append(E)
                for kt in range(KT):
                    for j2 in range(2):
                        h = half * 4 + hp * 2 + j2
                        nc.tensor.matmul(
                            U4[hp][:, j2, 0:QB],
                            lhsT=v2e[g * KT + kt][:, h, :],
                            rhs=Es[kt][:, j2, :],
                            start=(kt == 0),
                            stop=(kt == KT - 1),
                        )
            for t in range(2):
                rr = dp.tile([64, 2, QB], F32, tag=f"rr{t}", name=f"rr{t}")
                nc.vector.reciprocal(out=rr, in_=U4[t][:, :, 0:QB])
                for j2 in range(2):
                    j = t * 2 + j2
                    nc.vector.scalar_tensor_tensor(
                        out=ctx4[half][32 * j : 32 * j + 32, g * QB : (g + 1) * QB],
                        in0=U4[t][0:32, j2, 0:QB],
                        scalar=0.0,
                        in1=rr[32:64, j2, :],
                        op0=ALU.bypass,
                        op1=ALU.mult,
                    )

        def _outproj(qt, lp):
            ps = sp.tile([128, QD], F32, tag="S")
            nc.tensor.matmul(
                ps[0:1, 0:1],
                lhsT=ctx4[1][0:1, 128 * qt : 128 * qt + 1].bitcast(F32),
                rhs=epst[0:1, 0:1],
                start=True,
                stop=True,
                skip_group_check=True,
            )
            for t in range(2):
                nc.tensor.matmul(
                    ps,
                    lhsT=ctx4[t][:, 128 * qt : 128 * (qt + 1)],
                    rhs=wo4[:, t, :],
                    start=(t == 0),
                    stop=(t == 1),
                )
            x = lp.tile([128, QD], F32, tag="x")
            nc.vector.tensor_add(x, ps, xqtok[:, qt, :])
            stats = lp.tile([128, 6], F32, tag="stats")
            nc.vector.bn_stats(out=stats, in_=x)
            mv = lp.tile([128, 2], F32, tag="mv")
            nc.vector.bn_aggr(out=mv, in_=stats)
            lv = lp.tile([128, 1], F32, tag="lv")
            nc.scalar.activation(
                out=lv, in_=mv[:, 1:2], func=AF.Ln, bias=epst[:, 0:1]
            )
            rstd = lp.tile([128, 1], F32, tag="rstd")
            nc.scalar.activation(out=rstd, in_=lv, func=AF.Exp, scale=-0.5)
            xc = lp.tile([128, QD], F32, tag="xc")
            nc.vector.tensor_scalar(
                out=xc,
                in0=x,
                scalar1=mv[:, 0:1],
                scalar2=None,
                op0=ALU.subtract,
            )
            y = lp.tile([128, QD], F32, tag="y")
            nc.vector.scalar_tensor_tensor(
                out=y,
                in0=xc,
                scalar=rstd[:, 0:1],
                in1=lnt[:, 0, :],
                op0=ALU.mult,
                op1=ALU.mult,
            )
            yb = lp.tile([128, QD], F32, tag="yb")
            nc.gpsimd.tensor_add(yb, y, lnt[:, 1, :])
            nc.sync.dma_start(out=out_d[128 * qt : 128 * (qt + 1), :], in_=yb)

        # ---- emission: all projections (graph 0 first), then attention ----
        for mc in (0, 1):
            _proj_q(mc, 0)
            _proj_k(mc, 0)
        for kt in range(KT):
            _proj_v(kt)
        _proj_q(2, 0)
        _proj_k(2, 0)
        for mc in range(3):
            _proj_q(mc, 1)
            _proj_k(mc, 1)
        for kt in range(KT, 2 * KT):
            _proj_v(kt)
        pp_cm.__exit__(None, None, None)

        sp_cm = tc.tile_pool(name="s_ps", bufs=2, space="PSUM")
        sp = sp_cm.__enter__()
        up_cm = tc.tile_pool(name="u_ps", bufs=1, space="PSUM")
        up = up_cm.__enter__()
        ep_cm = tc.tile_pool(name="e_sb", bufs=3)
        ep = ep_cm.__enter__()
        dp_cm = tc.tile_pool(name="d_sb", bufs=2)
        dp = dp_cm.__enter__()
        lp_cm = tc.tile_pool(name="ln_sb", bufs=3)
        lp = lp_cm.__enter__()

        _pass(0)
        _pass(1)
        g0_qts = list(range(QB // 128))  # token tiles fully inside graph 0
        for qt in g0_qts:
            _outproj(qt, lp)
        _pass(2)
        _pass(3)
        for qt in range(len(g0_qts), QT):
            _outproj(qt, lp)

        lp_cm.__exit__(None, None, None)
        dp_cm.__exit__(None, None, None)
        ep_cm.__exit__(None, None, None)
        up_cm.__exit__(None, None, None)
        sp_cm.__exit__(None, None, None)
        persist_cm.__exit__(None, None, None)

    nc.compile()
    return nc


def kernel(**inputs):
    xq = np.ascontiguousarray(np.asarray(inputs["query_nodes"], dtype=np.float32))
    xk = np.ascontiguousarray(np.asarray(inputs["key_nodes"], dtype=np.float32))
    qbi = np.asarray(inputs["query_batch_idx"]).astype(np.int64)
    kbi = np.asarray(inputs["key_batch_idx"]).astype(np.int64)
    Wq = np.asarray(inputs["Wq"], np.float32)
    Wk = np.asarray(inputs["Wk"], np.float32)
    Wv = np.asarray(inputs["Wv"], np.float32)
    bq0 = np.asarray(inputs["bq"], np.float32)
    bk0 = np.asarray(inputs["bk"], np.float32)
    bv0 = np.asarray(inputs["bv"], np.float32)
    W2 = np.asarray(inputs["in_proj_w"], np.float32)
    b2 = np.asarray(inputs["in_proj_b"], np.float32)
    mow = np.asarray(inputs["mha_ow"], np.float32)
    mob = np.asarray(inputs["mha_ob"], np.float32)
    Wo = np.asarray(inputs["Wo"], np.float32)
    bo = np.asarray(inputs["bo"], np.float32)
    lng = np.asarray(inputs["ln_g"], np.float32)
    lnb = np.asarray(inputs["ln_b"], np.float32)

    # host-side weight folding
    Wq_eff = (Wq @ W2[:HID].T) * SCALE
    bq_eff = (bq0 @ W2[:HID].T + b2[:HID]) * SCALE
    Wk_eff = Wk @ W2[HID : 2 * HID].T
    bk_eff = bk0 @ W2[HID : 2 * HID].T + b2[HID : 2 * HID]
    Wv_eff = Wv @ W2[2 * HID :].T
    bv_eff = bv0 @ W2[2 * HID :].T + b2[2 * HID :]
    Wout_eff = mow @ Wo
    bout = bv_eff @ Wout_eff + mob @ Wo + bo  # folded into residual

    qcnt = np.bincount(qbi, minlength=NB)
    kcnt = np.bincount(kbi, minlength=NB)
    qoff = np.concatenate([[0], np.cumsum(qcnt)])
    koff = np.concatenate([[0], np.cumsum(kcnt)])

    QB = int(_ceil(max(int(qcnt.max()), 256), 8) * 8)
    KB = int(_ceil(max(int(kcnt.max()), 1), 128) * 128)
    NQC = GPC * QB
    NQCP = _ceil(NQC, 128) * 128
    KBC = GPC * KB

    nc = _build_program(QB, KB, NQC, NQCP, KBC)

    # pack 8 heads as 3-per-128-partition-tile (PE base-partition must be 0/32/64)
    def _headpack_cols(W):
        Wp = np.zeros((W.shape[0], 384), np.float32)
        for h in range(NH):
            Wp[:, 128 * (h // 3) + DH * (h % 3) : 128 * (h // 3) + DH * (h % 3) + DH] = (
                W[:, DH * h : DH * (h + 1)]
            )
        return Wp

    def _headpack_vec(v):
        vp = np.zeros((384,), np.float32)
        for h in range(NH):
            vp[128 * (h // 3) + DH * (h % 3) : 128 * (h // 3) + DH * (h % 3) + DH] = v[
                DH * h : DH * (h + 1)
            ]
        return vp

    def _chunk_rows(M, nchunk):
        # [R, C] -> [128, nchunk*C]: row chunk c side by side (short chunk zero-padded)
        R, C = M.shape
        out = np.zeros((128, nchunk * C), np.float32)
        for c in range(nchunk):
            a, b = 128 * c, min(128 * (c + 1), R)
            out[0 : b - a, c * C : c * C + C] = M[a:b]
        return out

    wqT = _headpack_cols(Wq_eff)  # [256, 384]
    wkT = np.zeros((KD + 1, 384), np.float32)
    wkT[:KD] = _headpack_cols(Wk_eff)
    wvT = np.zeros((KD + 1, NH * 2 * DH), np.float32)
    for h in range(NH):
        wvT[:KD, 2 * DH * h : 2 * DH * h + DH] = Wv_eff[:, DH * h : DH * (h + 1)]
        wvT[KD, 2 * DH * h + DH : 2 * DH * (h + 1)] = 1.0

    wq_p = _chunk_rows(wqT, 2)
    wk_p = _chunk_rows(wkT, 3)
    wv_p = _chunk_rows(wvT, 3)
    wo_p = _chunk_rows(np.ascontiguousarray(Wout_eff), 2)
    bq_eff = _headpack_vec(bq_eff)
    bk_eff = _headpack_vec(bk_eff)
    bqk_p = np.zeros((128, 6), np.float32)
    for c in range(3):
        bqk_p[:, c] = bq_eff[128 * c : 128 * (c + 1)]
        bqk_p[:, 3 + c] = bk_eff[128 * c : 128 * (c + 1)]
    ln_p = np.concatenate([lng, lnb])

    QT = NQCP // 128
    in_maps = []
    for c in range(NCORES):
        xqT = np.zeros((QD, NQC), np.float32)
        xqtok = np.zeros((NQCP, QD), np.float32)
        xkT = np.zeros((KD + 1, KBC), np.float32)
        for gi in range(GPC):
            g = GPC * c + gi
            nq = int(qcnt[g])
            nk = int(kcnt[g])
            if nq:
                rows = xq[qoff[g] : qoff[g + 1]]
                xqT[:, gi * QB : gi * QB + nq] = rows.T
                xqtok[gi * QB : gi * QB + nq] = rows + bout
            if nk:
                xkT[:KD, gi * KB : gi * KB + nk] = xk[koff[g] : koff[g + 1]].T
                xkT[KD, gi * KB : gi * KB + nk] = 1.0
        xqtok_p = np.zeros((128, QT * QD), np.float32)
        for t in range(QT):
            xqtok_p[:, t * QD : (t + 1) * QD] = xqtok[128 * t : 128 * (t + 1)]
        in_maps.append(
            {
                "xq": _chunk_rows(xqT, 2),
                "xk": _chunk_rows(xkT, 3),
                "wq": wq_p,
                "wk": wk_p,
                "wv": wv_p,
                "wo": wo_p,
                "xqtok": xqtok_p,
                "bqk": bqk_p.copy(),
                "ln": ln_p.copy(),
            }
        )

    import os

    trace = bool(os.environ.get("BASS_TRACE"))
    res = run_bass_kernel_spmd(nc, in_maps, list(range(NCORES)), trace=trace)
    if getattr(res, "exec_time_ns", None):
        print(f"HW exec time: {res.exec_time_ns} ns")
    out = np.empty((NQ, QD), np.float32)
    for c in range(NCORES):
        oc = res.results[c]["out"]
        for gi in range(GPC):
            g = GPC * c + gi
            nq = int(qcnt[g])
            if nq:
                out[qoff[g] : qoff[g + 1]] = oc[gi * QB : gi * QB + nq]
    return out


# revision 22
# speedup vs baseline: 1.0510x; 1.0388x over previous
"""GuidedCrossAttention Trainium2 kernel.

Sharding: 16 graphs -> 8 cores, 2 graphs per core (sorted batch indices make
graphs contiguous row-ranges). Per core we run block-diagonal attention on its
two graphs only. All projections are host-folded into single effective
matrices:
  q2 = xq @ Wq_eff + bq_eff      (SCALE folded in)
  k2 = xk @ Wk_eff + bk_eff
  v2 = xk @ Wv_eff               (v-bias folded into the residual via out-proj)
  out = ctx @ Wout_eff (+ bout folded into the residual term on host)

Device pipeline (feature-major activations, float32r matmuls throughout):
  - all inputs packed host-side into one DMA per tensor, triggers spread
    over sync/scalar/vector/gpsimd so compute starts as soon as possible
  - per-(mc, graph) q2T/k2T tiles so attention on graph 0 starts while
    graph 1 still projects
  - v2 projected with per-head [V(32) | valid-replicated(32)] columns, so the
    U matmul drops both the unnormalized context AND a 32-row broadcast of
    the softmax denominator into PSUM
  - per pass (graph, half-of-heads): all S^T matmuls first, exp per key-tile
    on scalar engine, then the U accumulation chain -> tensor never waits on
    the scalar engine in steady state
  - normalization: one full-tile vector reciprocal per U-pair tile + one
    STT multiply per head into packed ctx4 tiles ([128, NQCP], 4 heads each)
  - out-projection: K=128 matmuls (2 per token tile) using ctx4 directly
  - LayerNorm rstd via exp(-0.5*ln(var+eps)) to stay in the exp/ln ACT table
    set (no mid-kernel table switch)
"""

import math

import numpy as np

import concourse.bass as bass
import concourse.tile as tile
from concourse import bacc, mybir
from concourse.bass_utils import run_bass_kernel_spmd

QD, KD, HID, NH = 256, 320, 256, 8
NQ, NK, NB = 4096, 4096, 16
DH = HID // NH
EPS = 1e-5
SCALE = 1.0 / math.sqrt(DH)
NCORES = 8
GPC = NB // NCORES  # graphs per core
F32 = mybir.dt.float32
F32R = mybir.dt.float32r  # fp32 data, single-pass PE mode (4x fp32 throughput)
AF = mybir.ActivationFunctionType
ALU = mybir.AluOpType


def _ceil(a, b):
    return -(-a // b)


def _build_program(QB, KB, NQC, NQCP, KBC):
    KT = KB // 128  # key tiles per graph
    QT = NQCP // 128  # token-major query tiles

    nc = bacc.Bacc(
        "TRN2", target_bir_lowering=False, debug=False, num_devices=NCORES
    )
    xq_d = nc.declare_dram_parameter("xq", [128, 2 * NQC], F32R, isOutput=False)
    xk_d = nc.declare_dram_parameter("xk", [128, 3 * KBC], F32R, isOutput=False)
    wq_d = nc.declare_dram_parameter("wq", [128, 2 * 384], F32R, isOutput=False)
    wk_d = nc.declare_dram_parameter("wk", [128, 3 * 384], F32R, isOutput=False)
    wv_d = nc.declare_dram_parameter("wv", [128, 3 * NH * 2 * DH], F32R, isOutput=False)
    wo_d = nc.declare_dram_parameter("wo", [128, 2 * QD], F32R, isOutput=False)
    xqtok_d = nc.declare_dram_parameter("xqtok", [128, QT * QD], F32, isOutput=False)
    bqk_d = nc.declare_dram_parameter("bqk", [128, 6], F32, isOutput=False)
    ln_d = nc.declare_dram_parameter("ln", [2 * QD], F32, isOutput=False)
    out_d = nc.declare_dram_parameter("out", [NQCP, QD], F32, isOutput=True)

    kchunks = [(0, 128), (128, 256), (256, KD + 1)]  # xk row chunks (last: 65)

    with tile.TileContext(nc) as tc:
        persist_cm = tc.tile_pool(name="persist", bufs=1)
        P = persist_cm.__enter__()

        # ---- input loads: one DMA per tensor, spread across engines ----
        xqT = P.tile([128, 2, NQC], F32R, tag="xqT", name="xqT")
        wqt = P.tile([128, 2, 384], F32R, tag="wqt", name="wqt")
        bqk = P.tile([128, 6], F32, tag="bqk", name="bqk")
        nc.sync.dma_start(out=xqT, in_=xq_d[:, :])
        nc.sync.dma_start(out=wqt, in_=wq_d[:, :])
        nc.sync.dma_start(out=bqk, in_=bqk_d[:, :])
        xkT = P.tile([128, 3, KBC], F32R, tag="xkT", name="xkT")
        wkt = P.tile([128, 3, 384], F32R, tag="wkt", name="wkt")
        nc.scalar.dma_start(out=xkT, in_=xk_d[:, :])
        nc.scalar.dma_start(out=wkt, in_=wk_d[:, :])
        wvt = P.tile([128, 3, NH * 2 * DH], F32R, tag="wvt", name="wvt")
        lnt = P.tile([128, 2, QD], F32, tag="lnt", name="lnt")
        nc.scalar.dma_start(out=wvt, in_=wv_d[:, :])
        nc.gpsimd.dma_start(
            out=lnt,
            in_=bass.AP(tensor=ln_d.ap().tensor, offset=0, ap=[[0, 128], [1, 2 * QD]]),
        )
        wo4 = P.tile([128, 2, QD], F32R, tag="wo4", name="wo4")
        xqtok = P.tile([128, QT, QD], F32, tag="xqtok", name="xqtok")
        nc.gpsimd.dma_start(out=wo4, in_=wo_d[:, :])
        nc.gpsimd.dma_start(out=xqtok, in_=xqtok_d[:, :])

        epst = P.tile([128, 1], F32, tag="epst")
        nc.vector.memset(epst, EPS)

        # per (mc, graph) projection outputs; per key-tile v2e; packed ctx4
        q2T = [
            [P.tile([128, QB], F32R, tag=f"q2T{mc}{g}", name=f"q2T{mc}{g}") for g in range(GPC)]
            for mc in range(3)
        ]
        k2T = [
            [P.tile([128, KB], F32R, tag=f"k2T{mc}{g}", name=f"k2T{mc}{g}") for g in range(GPC)]
            for mc in range(3)
        ]
        v2e = [
            P.tile([128, NH, 2 * DH], F32R, tag=f"v2e{i}", name=f"v2e{i}")
            for i in range(GPC * KT)
        ]
        ctx4 = [P.tile([128, NQCP], F32R, tag=f"ctx4{i}", name=f"ctx4{i}") for i in range(2)]

        # batched LayerNorm stats + Quake-rsqrt work tiles ([128, QT] each)
        I32 = mybir.dt.int32
        mvall = P.tile([128, QT, 2], F32, tag="mvall", name="mvall")
        rstdall = P.tile([128, QT], F32, tag="rstdall", name="rstdall")
        rw = {
            n: P.tile([128, QT], F32, tag=f"rw_{n}", name=f"rw_{n}")
            for n in ("vi", "hv", "t1", "t2", "ya")
        }
        ih32 = P.tile([128, QT], I32, tag="rw_ih", name="rw_ih")
        magict = P.tile([128, QT], I32, tag="rw_magic", name="rw_magic")
        nc.vector.memset(magict, 0x5F3759DF)

        pp_cm = tc.tile_pool(name="proj_ps", bufs=2, space="PSUM")
        pp = pp_cm.__enter__()

        junked = set()

        def _junk(ps, *tiles):
            # 1x1 matmul reading each tile once: orders DMA completion
            # ahead of the next accumulation-group start
            for t in tiles:
                if id(t) in junked:
                    continue
                junked.add(id(t))
                src = t[tuple(slice(0, 1) for _ in range(len(t.shape)))]
                nc.tensor.matmul(
                    ps[0:1, 0:1],
                    lhsT=src.bitcast(F32) if src.dtype == F32R else src,
                    rhs=epst[0:1, 0:1],
                    start=True,
                    stop=True,
                    skip_group_check=True,
                )

        def _proj_q(mc, g):
            ps = pp.tile([128, 512], F32, tag="pj_ps")
            _junk(ps, xqT, wqt)
            for kc in range(2):
                nc.tensor.matmul(
                    ps[:, 0:QB],
                    lhsT=wqt[:, kc, 128 * mc : 128 * (mc + 1)],
                    rhs=xqT[:, kc, g * QB : (g + 1) * QB],
                    start=(kc == 0),
                    stop=(kc == 1),
                )
            nc.vector.tensor_scalar(
                out=q2T[mc][g],
                in0=ps[:, 0:QB],
                scalar1=bqk[:, mc : mc + 1],
                scalar2=None,
                op0=ALU.add,
            )

        def _proj_k(mc, g):
            ps = pp.tile([128, 512], F32, tag="pj_ps")
            _junk(ps, xkT, wkt)
            for kc, (a, b) in enumerate(kchunks):
                nc.tensor.matmul(
                    ps[:, 0:KB],
                    lhsT=wkt[0 : b - a, kc, 128 * mc : 128 * (mc + 1)],
                    rhs=xkT[0 : b - a, kc, g * KB : (g + 1) * KB],
                    start=(kc == 0),
                    stop=(kc == 2),
                )
            nc.vector.tensor_scalar(
                out=k2T[mc][g],
                in0=ps[:, 0:KB],
                scalar1=bqk[:, 3 + mc : 4 + mc],
                scalar2=None,
                op0=ALU.add,
            )

        def _proj_v(kt):
            ps = pp.tile([128, 512], F32, tag="pj_ps")
            _junk(ps, wvt)
            for kc, (a, b) in enumerate(kchunks):
                nc.tensor.matmul(
                    ps,
                    lhsT=xkT[0 : b - a, kc, 128 * kt : 128 * (kt + 1)],
                    rhs=wvt[0 : b - a, kc, :],
                    start=(kc == 0),
                    stop=(kc == 2),
                )
            nc.vector.tensor_copy(
                out=v2e[kt].rearrange("p h d -> p (h d)"),
                in_=ps,
            )

        passes = [(g, half) for g in range(GPC) for half in range(2)]

        def _pass(pi):
            g, half = passes[pi]
            U4 = [
                up.tile([64, 2, 512], F32, tag=f"U4{t}", name=f"U4{t}")
                for t in range(2)
            ]
            if pi > 0:
                pg, ph = passes[pi - 1]
                for t in range(2):
                    # WAR gate: new U group waits for prev pass's ctx4 writes
                    nc.tensor.matmul(
                        U4[t][0:1, 0, 0:1],
                        lhsT=ctx4[ph][0:1, pg * QB : pg * QB + 1].bitcast(F32),
                        rhs=epst[0:1, 0:1],
                        start=True,
                        stop=True,
                        skip_group_check=True,
                    )
            for hp in range(2):
                Es = []
                for kt in range(KT):
                    S = sp.tile([128, 2, 512], F32, tag="S")
                    for j2 in range(2):
                        h = half * 4 + hp * 2 + j2
                        mc, r = h // 3, (h % 3) * DH
                        nc.tensor.matmul(
                            S[:, j2, 0:QB],
                            lhsT=k2T[mc][g][r : r + DH, 128 * kt : 128 * (kt + 1)],
                            rhs=q2T[mc][g][r : r + DH, :],
                            start=True,
                            stop=True,
                        )
                    E = ep.tile([128, 2, QB], F32R, tag="E")
                    nc.scalar.activation(out=E, in_=S[:, :, 0:QB], func=AF.Exp)
                    Es.append(E)
                for kt in range(KT):
                    for j2 in range(2):
                        h = half * 4 + hp * 2 + j2
                        nc.tensor.matmul(
                            U4[hp][:, j2, 0:QB],
                            lhsT=v2e[g * KT + kt][:, h, :],
                            rhs=Es[kt][:, j2, :],
                            start=(kt == 0),
                            stop=(kt == KT - 1),
                        )
            for t in range(2):
                rr = dp.tile([64, 2, QB], F32, tag=f"rr{t}", name=f"rr{t}")
                nc.vector.reciprocal(out=rr, in_=U4[t][:, :, 0:QB])
                for j2 in range(2):
                    j = t * 2 + j2
                    nc.vector.scalar_tensor_tensor(
                        out=ctx4[half][32 * j : 32 * j + 32, g * QB : (g + 1) * QB],
                        in0=U4[t][0:32, j2, 0:QB],
                        scalar=0.0,
                        in1=rr[32:64, j2, :],
                        op0=ALU.bypass,
                        op1=ALU.mult,
                    )

        xts = {}

        def _outproj_head(qt, lp):
            ps = sp.tile([128, QD], F32, tag="S")
            nc.tensor.matmul(
                ps[0:1, 0:1],
                lhsT=ctx4[1][0:1, 128 * qt : 128 * qt + 1].bitcast(F32),
                rhs=epst[0:1, 0:1],
                start=True,
                stop=True,
                skip_group_check=True,
            )
            for t in range(2):
                nc.tensor.matmul(
                    ps,
                    lhsT=ctx4[t][:, 128 * qt : 128 * (qt + 1)],
                    rhs=wo4[:, t, :],
                    start=(t == 0),
                    stop=(t == 1),
                )
            x = lp.tile([128, QD], F32, tag=f"x{qt}", name=f"x{qt}")
            xts[qt] = x
            nc.vector.tensor_add(x, ps, xqtok[:, qt, :])
            stats = lp.tile([128, 6], F32, tag="stats")
            nc.vector.bn_stats(out=stats, in_=x)
            nc.vector.bn_aggr(out=mvall[:, qt, :], in_=stats)

        def _rstd_group(qts):
            # rstd = 1/sqrt(var+eps) via Quake seed + 2 Newton steps, batched
            sl = slice(qts[0], qts[-1] + 1)
            vi, hv, t1, t2, ya = (rw[n] for n in ("vi", "hv", "t1", "t2", "ya"))
            nc.vector.tensor_scalar(
                out=vi[:, sl], in0=mvall[:, sl, 1], scalar1=EPS, scalar2=None, op0=ALU.add
            )
            nc.vector.tensor_scalar(
                out=hv[:, sl], in0=vi[:, sl], scalar1=-0.5, scalar2=None, op0=ALU.mult
            )
            nc.vector.tensor_scalar(
                out=ih32[:, sl],
                in0=vi[:, sl].bitcast(I32),
                scalar1=1,
                scalar2=None,
                op0=ALU.arith_shift_right,
            )
            nc.vector.tensor_tensor(
                out=ya[:, sl].bitcast(I32), in0=magict[:, sl], in1=ih32[:, sl], op=ALU.subtract
            )
            # Newton iter 1: ya -> vi (vi's value is dead once hv/ih exist)
            nc.vector.tensor_tensor(out=t1[:, sl], in0=ya[:, sl], in1=ya[:, sl], op=ALU.mult)
            nc.vector.tensor_tensor(out=t2[:, sl], in0=t1[:, sl], in1=hv[:, sl], op=ALU.mult)
            nc.vector.tensor_scalar(
                out=t2[:, sl], in0=t2[:, sl], scalar1=1.5, scalar2=None, op0=ALU.add
            )
            nc.vector.tensor_tensor(out=vi[:, sl], in0=ya[:, sl], in1=t2[:, sl], op=ALU.mult)
            # Newton iter 2: vi -> rstdall
            nc.vector.tensor_tensor(out=t1[:, sl], in0=vi[:, sl], in1=vi[:, sl], op=ALU.mult)
            nc.vector.tensor_tensor(out=t2[:, sl], in0=t1[:, sl], in1=hv[:, sl], op=ALU.mult)
            nc.vector.tensor_scalar(
                out=t2[:, sl], in0=t2[:, sl], scalar1=1.5, scalar2=None, op0=ALU.add
            )
            nc.vector.tensor_tensor(out=rstdall[:, sl], in0=vi[:, sl], in1=t2[:, sl], op=ALU.mult)

        def _outproj_tail(qt, lp):
            x = xts[qt]
            xc = lp.tile([128, QD], F32, tag="xc")
            nc.vector.tensor_scalar(
                out=xc,
                in0=x,
                scalar1=mvall[:, qt, 0:1],
                scalar2=None,
                op0=ALU.subtract,
            )
            y = lp.tile([128, QD], F32, tag="y")
            nc.vector.scalar_tensor_tensor(
                out=y,
                in0=xc,
                scalar=rstdall[:, qt : qt + 1],
                in1=lnt[:, 0, :],
                op0=ALU.mult,
                op1=ALU.mult,
            )
            yb = lp.tile([128, QD], F32, tag="yb")
            nc.gpsimd.tensor_add(yb, y, lnt[:, 1, :])
            nc.sync.dma_start(out=out_d[128 * qt : 128 * (qt + 1), :], in_=yb)

        # ---- emission: all projections (graph 0 first), then attention ----
        for mc in (0, 1):
            _proj_q(mc, 0)
            _proj_k(mc, 0)
        for kt in range(KT):
            _proj_v(kt)
        _proj_q(2, 0)
        _proj_k(2, 0)
        for mc in range(3):
            _proj_q(mc, 1)
            _proj_k(mc, 1)
        for kt in range(KT, 2 * KT):
            _proj_v(kt)
        pp_cm.__exit__(None, None, None)

        sp_cm = tc.tile_pool(name="s_ps", bufs=2, space="PSUM")
        sp = sp_cm.__enter__()
        up_cm = tc.tile_pool(name="u_ps", bufs=1, space="PSUM")
        up = up_cm.__enter__()
        ep_cm = tc.tile_pool(name="e_sb", bufs=3)
        ep = ep_cm.__enter__()
        dp_cm = tc.tile_pool(name="d_sb", bufs=2)
        dp = dp_cm.__enter__()
        lp_cm = tc.tile_pool(name="ln_sb", bufs=3)
        lp = lp_cm.__enter__()

        _pass(0)
        _pass(1)
        g0_qts = list(range(QB // 128))  # token tiles fully inside graph 0
        for qt in g0_qts:
            _outproj_head(qt, lp)
        _rstd_group(g0_qts)
        for qt in g0_qts:
            _outproj_tail(qt, lp)
        _pass(2)
        _pass(3)
        g1_qts = list(range(len(g0_qts), QT))
        for qt in g1_qts:
            _outproj_head(qt, lp)
        _rstd_group(g1_qts)
        for qt in g1_qts:
            _outproj_tail(qt, lp)

        lp_cm.__exit__(None, None, None)
        dp_cm.__exit__(None, None, None)
        ep_cm.__exit__(None, None, None)
        up_cm.__exit__(None, None, None)
        sp_cm.__exit__(None, None, None)
        persist_cm.__exit__(None, None, None)

    nc.compile()
    return nc


def kernel(**inputs):
    xq = np.ascontiguousarray(np.asarray(inputs["query_nodes"], dtype=np.float32))
    xk = np.ascontiguousarray(np.asarray(inputs["key_nodes"], dtype=np.float32))
    qbi = np.asarray(inputs["query_batch_idx"]).astype(np.int64)
    kbi = np.asarray(inputs["key_batch_idx"]).astype(np.int64)
    Wq = np.asarray(inputs["Wq"], np.float32)
    Wk = np.asarray(inputs["Wk"], np.float32)
    Wv = np.asarray(inputs["Wv"], np.float32)
    bq0 = np.asarray(inputs["bq"], np.float32)
    bk0 = np.asarray(inputs["bk"], np.float32)
    bv0 = np.asarray(inputs["bv"], np.float32)
    W2 = np.asarray(inputs["in_proj_w"], np.float32)
    b2 = np.asarray(inputs["in_proj_b"], np.float32)
    mow = np.asarray(inputs["mha_ow"], np.float32)
    mob = np.asarray(inputs["mha_ob"], np.float32)
    Wo = np.asarray(inputs["Wo"], np.float32)
    bo = np.asarray(inputs["bo"], np.float32)
    lng = np.asarray(inputs["ln_g"], np.float32)
    lnb = np.asarray(inputs["ln_b"], np.float32)

    # host-side weight folding
    Wq_eff = (Wq @ W2[:HID].T) * SCALE
    bq_eff = (bq0 @ W2[:HID].T + b2[:HID]) * SCALE
    Wk_eff = Wk @ W2[HID : 2 * HID].T
    bk_eff = bk0 @ W2[HID : 2 * HID].T + b2[HID : 2 * HID]
    Wv_eff = Wv @ W2[2 * HID :].T
    bv_eff = bv0 @ W2[2 * HID :].T + b2[2 * HID :]
    Wout_eff = mow @ Wo
    bout = bv_eff @ Wout_eff + mob @ Wo + bo  # folded into residual

    qcnt = np.bincount(qbi, minlength=NB)
    kcnt = np.bincount(kbi, minlength=NB)
    qoff = np.concatenate([[0], np.cumsum(qcnt)])
    koff = np.concatenate([[0], np.cumsum(kcnt)])

    QB = int(_ceil(max(int(qcnt.max()), 256), 8) * 8)
    KB = int(_ceil(max(int(kcnt.max()), 1), 128) * 128)
    NQC = GPC * QB
    NQCP = _ceil(NQC, 128) * 128
    KBC = GPC * KB

    nc = _build_program(QB, KB, NQC, NQCP, KBC)

    # pack 8 heads as 3-per-128-partition-tile (PE base-partition must be 0/32/64)
    def _headpack_cols(W):
        Wp = np.zeros((W.shape[0], 384), np.float32)
        for h in range(NH):
            Wp[:, 128 * (h // 3) + DH * (h % 3) : 128 * (h // 3) + DH * (h % 3) + DH] = (
                W[:, DH * h : DH * (h + 1)]
            )
        return Wp

    def _headpack_vec(v):
        vp = np.zeros((384,), np.float32)
        for h in range(NH):
            vp[128 * (h // 3) + DH * (h % 3) : 128 * (h // 3) + DH * (h % 3) + DH] = v[
                DH * h : DH * (h + 1)
            ]
        return vp

    def _chunk_rows(M, nchunk):
        # [R, C] -> [128, nchunk*C]: row chunk c side by side (short chunk zero-padded)
        R, C = M.shape
        out = np.zeros((128, nchunk * C), np.float32)
        for c in range(nchunk):
            a, b = 128 * c, min(128 * (c + 1), R)
            out[0 : b - a, c * C : c * C + C] = M[a:b]
        return out

    wqT = _headpack_cols(Wq_eff)  # [256, 384]
    wkT = np.zeros((KD + 1, 384), np.float32)
    wkT[:KD] = _headpack_cols(Wk_eff)
    wvT = np.zeros((KD + 1, NH * 2 * DH), np.float32)
    for h in range(NH):
        wvT[:KD, 2 * DH * h : 2 * DH * h + DH] = Wv_eff[:, DH * h : DH * (h + 1)]
        wvT[KD, 2 * DH * h + DH : 2 * DH * (h + 1)] = 1.0

    wq_p = _chunk_rows(wqT, 2)
    wk_p = _chunk_rows(wkT, 3)
    wv_p = _chunk_rows(wvT, 3)
    wo_p = _chunk_rows(np.ascontiguousarray(Wout_eff), 2)
    bq_eff = _headpack_vec(bq_eff)
    bk_eff = _headpack_vec(bk_eff)
    bqk_p = np.zeros((128, 6), np.float32)
    for c in range(3):
        bqk_p[:, c] = bq_eff[128 * c : 128 * (c + 1)]
        bqk_p[:, 3 + c] = bk_eff[128 * c : 128 * (c + 1)]
    ln_p = np.concatenate([lng, lnb])

    QT = NQCP // 128
    in_maps = []
    for c in range(NCORES):
        xqT = np.zeros((QD, NQC), np.float32)
        xqtok = np.zeros((NQCP, QD), np.float32)
        xkT = np.zeros((KD + 1, KBC), np.float32)
        for gi in range(GPC):
            g = GPC * c + gi
            nq = int(qcnt[g])
            nk = int(kcnt[g])
            if nq:
                rows = xq[qoff[g] : qoff[g + 1]]
                xqT[:, gi * QB : gi * QB + nq] = rows.T
                xqtok[gi * QB : gi * QB + nq] = rows + bout
            if nk:
                xkT[:KD, gi * KB : gi * KB + nk] = xk[koff[g] : koff[g + 1]].T
                xkT[KD, gi * KB : gi * KB + nk] = 1.0
        xqtok_p = np.zeros((128, QT * QD), np.float32)
        for t in range(QT):
            xqtok_p[:, t * QD : (t + 1) * QD] = xqtok[128 * t : 128 * (t + 1)]
        in_maps.append(
            {
                "xq": _chunk_rows(xqT, 2),
                "xk": _chunk_rows(xkT, 3),
                "wq": wq_p,
                "wk": wk_p,
                "wv": wv_p,
                "wo": wo_p,
                "xqtok": xqtok_p,
                "bqk": bqk_p.copy(),
                "ln": ln_p.copy(),
            }
        )

    import os

    trace = bool(os.environ.get("BASS_TRACE"))
    res = run_bass_kernel_spmd(nc, in_maps, list(range(NCORES)), trace=trace)
    if getattr(res, "exec_time_ns", None):
        print(f"HW exec time: {res.exec_time_ns} ns")
    out = np.empty((NQ, QD), np.float32)
    for c in range(NCORES):
        oc = res.results[c]["out"]
        for gi in range(GPC):
            g = GPC * c + gi
            nq = int(qcnt[g])
            if nq:
                out[qoff[g] : qoff[g + 1]] = oc[gi * QB : gi * QB + nq]
    return out
